# revision 4
# baseline (speedup 1.0000x reference)
"""MoE (DeepSeek-style) routed+shared expert forward on 8 TRN2 NeuronCores.

Strategy (expert-parallel, host-side dispatch):
  - Host computes the gate (softmax + top-2) in float64 and gathers each
    expert's routed tokens (padded to a uniform capacity C2 = 2048,
    capacity factor 1.0; the handful of over-capacity pairs are computed
    exactly on the host and scattered in with the rest).
  - Core e runs expert e's routed tokens through the SwiGLU FFN in
    fp8(e4m3) with DoubleRow matmuls (2 k-tiles per instruction), plus a
    1/8 slice of all tokens through the replicated shared-expert MLP in
    fp16.
  - Weights are scaled by SW into e4m3; the activation instructions
    de-scale via their `scale` operand. h is stored fp8 so the w2 matmul
    also runs in DoubleRow mode.
  - One dma_start lands on one of the 16 DMA engines (~45 GB/s each), so
    inputs are split into ~128-512 KB single-writer chunk tiles issued in
    exact consumption order.  The critical first tiles alternate between
    the two HW-DGE trigger rings; everything else streams on the sync
    ring only, so the scalar sequencer dispatches the SILUs (which drain
    PSUM) without queueing behind descriptor writes.
  - Host scatters expert outputs back by routing index, scales by the
    gate weights, and adds the shared-expert output.
"""

import sys

if "/opt/trn_rl_repo" not in sys.path:
    sys.path.insert(0, "/opt/trn_rl_repo")

import ml_dtypes
import numpy as np

import concourse.bass as bass
import concourse.tile as tile
from concourse import bacc, mybir
from concourse import bass_utils

B, S, DIM = 4, 2048, 1024
T = B * S
INTER = 1024
E = 8
TOPK = 2
ROUTE_SCALE = 1.0
SHARED_INTER = 2048
N_CORES = 8
TS = T // N_CORES   # shared-expert tokens per core
SW = 16.0           # weight scale into e4m3; h8 = silu(z1)*(SW*z3) stays
                    # under e4m3 max 240 (measured |16h| <= ~102)

F32 = mybir.dt.float32
F16 = mybir.dt.float16
F8 = mybir.dt.float8e4
SILU = mybir.ActivationFunctionType.Silu
IDENT = mybir.ActivationFunctionType.Identity
DR = mybir.MatmulPerfMode.DoubleRow

_program_cache = {}


def build_program(C2, BS, NB):
    """Per-core SPMD Bass program. C2 = NB*BS routed capacity."""
    assert 2 <= NB <= 8, f"dr_chain streams NB blocks over an 8-bank PSUM ring, {NB=}"
    nc = bacc.Bacc("TRN2", target_bir_lowering=False, debug=False,
                   num_devices=N_CORES)

    def din(name, shape, dt):
        return nc.dram_tensor(name, shape, dt, kind="ExternalInput").ap()

    def dout(name, shape, dt):
        return nc.dram_tensor(name, shape, dt, kind="ExternalOutput").ap()

    ND = DIM // 128           # 8 k-tiles over DIM
    NI = INTER // 128
    NS = SHARED_INTER // 128  # 16
    NP = ND // 2              # k-tile pairs for DoubleRow
    H1 = INTER // 2

    # All inputs are host-packed in SBUF layout (partition dim first,
    # per-partition data contiguous) so each DMA is 128 large contiguous
    # descriptors instead of thousands of sub-KB ones.
    xe_r = din("xe8", (128, NB, ND, BS), F8)   # routed tokens
    w1_r = din("w1t8", (128, NP, 2, INTER), F8)
    w3_r = din("w3t8", (128, NP, 2, INTER), F8)
    w2_r = din("w2t8", (128, NP, 2, DIM), F8)
    xs_r = din("xs", (128, ND, TS), F16)       # shared-token slice
    ws1_r = din("ws1t", (128, ND, SHARED_INTER), F16)
    ws3_r = din("ws3t", (128, ND, SHARED_INTER), F16)
    ws2_r = din("ws2t", (128, NS, DIM), F16)
    biases = din("biases", (128, 64), F32)     # host-packed per-partition
    ye = dout("ye", (DIM, C2), F16)
    ys = dout("ys", (DIM, TS), F16)

    ye_r = ye.rearrange("(md p) c -> p md c", p=128)
    ys_r = ys.rearrange("(md p) c -> p md c", p=128)

    with tile.TileContext(nc) as tc:
        from contextlib import ExitStack
        es1 = ExitStack()
        with tc.tile_pool(name="bias", bufs=1) as bpool, \
             tc.tile_pool(name="wsh", bufs=1, side="right") as wspool, \
             tc.tile_pool(name="tmp", bufs=NB + 2) as tpool, \
             tc.tile_pool(name="yout", bufs=NB + 2) as ypool, \
             tc.tile_pool(name="ps", bufs=8, space="PSUM") as pspool:

            wpool = es1.enter_context(tc.tile_pool(name="wexp", bufs=1))
            xpool = es1.enter_context(tc.tile_pool(name="xep", bufs=1))
            hpool = es1.enter_context(tc.tile_pool(name="h8p", bufs=1))

            # ---- PE pre-warm: the HAM clock gate holds the PE at 1.2 GHz
            # until ~3.4us of sustained activity.  Run dummy matmuls on a
            # memset tile while the first real inputs stream in. ----
            warm = bpool.tile([128, 640], F8, tag="warm")
            nc.vector.memset(warm[:], 0)
            tw = bpool.tile([128, 16], F16, tag="tw")
            for i in range(8):
                pw = pspool.tile([128, 512], F32, tag="ps", name="ps",
                                 padded_shape=[128, 512])
                n = 512 if i < 6 else 128
                nc.tensor.matmul(pw[:, 0:n], warm[:, 0:128],
                                 warm[:, 128:128 + n],
                                 start=True, stop=True)

            ball = bpool.tile([128, 64], F32, tag="biases")
            b1_sb = ball[:, 0:NI]
            b2_sb = ball[:, 2 * NI:2 * NI + ND]
            bs1_sb = ball[:, 24:24 + NS]
            bs2_sb = ball[:, 24 + 2 * NS:24 + 2 * NS + ND]

            rings = [nc.sync, nc.scalar]
            ring_i = [0]

            def dma(dst, srcap):
                rings[ring_i[0] % 2].dma_start(dst, srcap)
                ring_i[0] += 1

            # ---- single-writer chunk tiles (one dma_start per tile) ----
            # w1/w3: per (j, colhalf): [128, 2, 512];  lhsT for (mi, j) is
            # w[j][mi//4][:, :, (mi%4)*128:...].
            w1_sb = [[wpool.tile([128, 2, H1], F8, tag=f"w1_{j}_{c}",
                                 name=f"w1_{j}_{c}") for c in range(2)]
                     for j in range(NP)]
            w3_sb = [[wpool.tile([128, 2, H1], F8, tag=f"w3_{j}_{c}",
                                 name=f"w3_{j}_{c}") for c in range(2)]
                     for j in range(NP)]
            w2_sb = [[wpool.tile([128, 2, H1], F8, tag=f"w2_{j}_{c}",
                                 name=f"w2_{j}_{c}") for c in range(2)]
                     for j in range(NP)]
            # xe: per (b, j): [128, 2, BS] — exactly one DoubleRow rhs.
            xe_sb = [[xpool.tile([128, 2, BS], F8, tag=f"xe{b}_{j}",
                                 name=f"xe{b}_{j}") for j in range(NP)]
                     for b in range(NB)]

            def w_of(w, mi, j):
                return w[j][mi // 4][:, :, (mi % 4) * 128:(mi % 4 + 1) * 128]

            # Critical preamble, in consumption order, alternating rings.
            # Scalar ring gets the biases + a bounded number of chunks so
            # the first SILU dispatches before PSUM fills (~8 bank-pairs).
            nc.scalar.dma_start(ball[:], biases[:])
            for j in range(NP):
                dma(w1_sb[j][0][:], w1_r[:, j, :, 0:H1])
            for j in range(NP):
                dma(xe_sb[0][j][:], xe_r[:, 0, 2 * j:2 * j + 2, :])
            # ACT tables: preload on the scalar engine before first SILU.
            nc.scalar.activation(tw[:], warm[:, 0:16], SILU)
            nc.scalar.activation(tw[:], warm[:, 0:16], IDENT)
            for j in range(NP):
                dma(w3_sb[j][0][:], w3_r[:, j, :, 0:H1])
            for j in range(NP):
                dma(xe_sb[1][j][:], xe_r[:, 1, 2 * j:2 * j + 2, :])

            # Everything else streams on the sync ring only, in
            # consumption order; the scalar sequencer stays free for the
            # SILUs that drain PSUM.
            for b in range(2, NB):
                for j in range(NP):
                    nc.sync.dma_start(xe_sb[b][j][:],
                                      xe_r[:, b, 2 * j:2 * j + 2, :])
            for j in range(NP):
                nc.sync.dma_start(w1_sb[j][1][:], w1_r[:, j, :, H1:INTER])
            for j in range(NP):
                nc.sync.dma_start(w3_sb[j][1][:], w3_r[:, j, :, H1:INTER])
            for c in range(2):
                for j in range(NP):
                    nc.sync.dma_start(w2_sb[j][c][:],
                                      w2_r[:, j, :, c * H1:(c + 1) * H1])

            # ---- phase-2 inputs: single-writer chunk tiles on sync ----
            xs_sb = [wspool.tile([128, 2, TS], F16, tag=f"xs{q}",
                                 name=f"xs{q}") for q in range(ND // 2)]
            for q in range(ND // 2):
                nc.sync.dma_start(xs_sb[q][:], xs_r[:, 2 * q:2 * q + 2, :])
            ws1_sb = [wspool.tile([128, 1, SHARED_INTER], F16, tag=f"ws1_{k}",
                                  name=f"ws1_{k}") for k in range(ND)]
            ws3_sb = [wspool.tile([128, 1, SHARED_INTER], F16, tag=f"ws3_{k}",
                                  name=f"ws3_{k}") for k in range(ND)]
            for k in range(ND):
                nc.sync.dma_start(ws1_sb[k][:], ws1_r[:, k:k + 1, :])
            for k in range(ND):
                nc.sync.dma_start(ws3_sb[k][:], ws3_r[:, k:k + 1, :])
            ws2_sb = [wspool.tile([128, 4, DIM], F16, tag=f"ws2_{q}",
                                  name=f"ws2_{q}") for q in range(NS // 4)]
            for q in range(NS // 4):
                nc.sync.dma_start(ws2_sb[q][:], ws2_r[:, 4 * q:4 * q + 4, :])

            h_sb = [hpool.tile([128, NI, BS], F8, tag=f"h{b}", name=f"h{b}")
                    for b in range(NB)]

            # ================= Phase 1: routed expert (fp8 DoubleRow) ====
            for mi in range(NI):
                # Per-block z1 -> silu -> z3 -> mul: spreads the early
                # xe-block DMA demand twice as thin as z1-for-all-blocks
                # first, so the head streams without stalling the PE.
                for b in range(NB):
                    ps1 = pspool.tile([128, BS], F32, tag="ps", name="ps",
                                      padded_shape=[128, 512])
                    for j in range(NP):
                        nc.tensor.matmul(
                            ps1[:], w_of(w1_sb, mi, j), xe_sb[b][j][:],
                            start=(j == 0), stop=(j == NP - 1),
                            perf_mode=DR)
                    t1 = tpool.tile([128, BS], F16, tag="t1",
                                    padded_shape=[128, 512])
                    nc.scalar.activation(t1[:], ps1[:], SILU,
                                         bias=b1_sb[:, mi:mi + 1],
                                         scale=1.0 / SW)
                    ps3 = pspool.tile([128, BS], F32, tag="ps", name="ps",
                                      padded_shape=[128, 512])
                    for j in range(NP):
                        nc.tensor.matmul(
                            ps3[:], w_of(w3_sb, mi, j), xe_sb[b][j][:],
                            start=(j == 0), stop=(j == NP - 1),
                            perf_mode=DR)
                    # b3 is zero, so h8 = t1 * (SW*z3) reads PSUM directly
                    nc.vector.tensor_mul(h_sb[b][:, mi, :], t1[:], ps3[:])

            for md in range(ND):
                # One weight load streams all NB blocks (j-outer).
                pss = [pspool.tile([128, BS], F32, tag="ps", name="ps",
                                   padded_shape=[128, 512])
                       for _ in range(NB)]
                for j in range(NP):
                    for b in range(NB):
                        nc.tensor.matmul(
                            pss[b][:], w_of(w2_sb, md, j),
                            h_sb[b][:, 2 * j:2 * j + 2, :],
                            start=(j == 0), stop=(j == NP - 1),
                            perf_mode=DR)
                for b in range(NB):
                    yt = ypool.tile([128, BS], F16, tag="yt",
                                    padded_shape=[128, 512])
                    nc.scalar.activation(yt[:], pss[b][:], IDENT,
                                         bias=b2_sb[:, md:md + 1],
                                         scale=1.0 / (SW * SW))
                    dma(ye_r[:, md, b * BS:(b + 1) * BS], yt[:])

            es1.close()  # free phase-1 pools; hsp reuses their space

            # ================= Phase 2: shared expert (fp16) =============
            NBS = 2
            BSS = TS // NBS  # 512
            hspool = ExitStack()
            hsp = hspool.enter_context(tc.tile_pool(name="hsp", bufs=1))
            hs_sb = [hsp.tile([128, NS, BSS], F16, tag=f"hs{b}", name=f"hs{b}")
                     for b in range(NBS)]

            def f16_chain(w_sb, msel, n):
                pss = [pspool.tile([128, n], F32, tag="ps", name="ps",
                                   padded_shape=[128, 512])
                       for _ in range(NBS)]
                for k in range(ND):
                    for b in range(NBS):
                        nc.tensor.matmul(
                            pss[b][:],
                            w_sb[k][:, 0, msel],
                            xs_sb[k // 2][:, k % 2, b * n:(b + 1) * n],
                            start=(k == 0), stop=(k == ND - 1))
                return pss

            for mi in range(NS):
                msel = slice(mi * 128, (mi + 1) * 128)
                ps1 = f16_chain(ws1_sb, msel, BSS)
                t1s = []
                for b in range(NBS):
                    t1 = tpool.tile([128, BSS], F16, tag="t1",
                                    padded_shape=[128, 512])
                    nc.scalar.activation(t1[:], ps1[b][:], SILU,
                                         bias=bs1_sb[:, mi:mi + 1])
                    t1s.append(t1)
                ps3 = f16_chain(ws3_sb, msel, BSS)
                for b in range(NBS):
                    nc.vector.tensor_mul(hs_sb[b][:, mi, :], t1s[b][:],
                                         ps3[b][:])

            for md in range(ND):
                msel = slice(md * 128, (md + 1) * 128)
                pss = [pspool.tile([128, BSS], F32, tag="ps", name="ps",
                                   padded_shape=[128, 512])
                       for _ in range(NBS)]
                for ji in range(NS):
                    for b in range(NBS):
                        nc.tensor.matmul(
                            pss[b][:],
                            ws2_sb[ji // 4][:, ji % 4, msel],
                            hs_sb[b][:, ji, :],
                            start=(ji == 0), stop=(ji == NS - 1))
                for b in range(NBS):
                    yt = ypool.tile([128, BSS], F16, tag="yt",
                                    padded_shape=[128, 512])
                    nc.scalar.activation(yt[:], pss[b][:], IDENT,
                                         bias=bs2_sb[:, md:md + 1])
                    # Split the store so the tail-critical final store
                    # lands on two engines instead of one.
                    hb = BSS // 2
                    dma(ys_r[:, md, b * BSS:b * BSS + hb], yt[:, 0:hb])
                    dma(ys_r[:, md, b * BSS + hb:(b + 1) * BSS], yt[:, hb:])
            hspool.close()

    nc.compile()
    return nc


def _pack_biases(b1, b3, b2, bs1, bs3, bs2):
    """Pack all bias vectors into one [128, 64] per-partition table."""
    out = np.zeros((128, 64), np.float32)
    cols = [(b1, 0), (b3, 8), (b2, 16), (bs1, 24), (bs3, 40),
            (bs2, 56)]
    for vec, c0 in cols:
        k = len(vec) // 128
        out[:, c0:c0 + k] = vec.reshape(k, 128).T
    return out


def _gate_host(xt, gate_w, gate_b):
    """Softmax gate + top-2 routing, computed in float64 on the host."""
    logits = xt.astype(np.float64) @ gate_w.astype(np.float64).T \
        + gate_b.astype(np.float64)
    m = logits.max(axis=-1, keepdims=True)
    p = np.exp(logits - m)
    scores = p / p.sum(axis=-1, keepdims=True)
    order = np.argsort(-scores, axis=1, kind="stable")
    top_i = order[:, :TOPK]
    top_w = (np.take_along_axis(scores, top_i, axis=1)
             * ROUTE_SCALE).astype(np.float32)
    return top_i, top_w


def _silu(z):
    return z / (1.0 + np.exp(-z))


def run(inputs, trace=False):
    f8 = ml_dtypes.float8_e4m3   # TRN-style e4m3 (max 240)
    f16 = np.float16

    x = np.ascontiguousarray(np.asarray(inputs["x"], dtype=np.float32))
    gate_w = np.asarray(inputs["gate_w"], dtype=np.float32)
    gate_b = np.asarray(inputs["gate_b"], dtype=np.float32)
    w1 = np.asarray(inputs["w1"], dtype=np.float32)
    b1 = np.asarray(inputs["b1"], dtype=np.float32)
    w3 = np.asarray(inputs["w3"], dtype=np.float32)
    b3 = np.asarray(inputs["b3"], dtype=np.float32)
    w2 = np.asarray(inputs["w2"], dtype=np.float32)
    b2 = np.asarray(inputs["b2"], dtype=np.float32)
    ws1 = np.asarray(inputs["ws1"], dtype=np.float32)
    bs1 = np.asarray(inputs["bs1"], dtype=np.float32)
    ws3 = np.asarray(inputs["ws3"], dtype=np.float32)
    bs3 = np.asarray(inputs["bs3"], dtype=np.float32)
    ws2 = np.asarray(inputs["ws2"], dtype=np.float32)
    bs2 = np.asarray(inputs["bs2"], dtype=np.float32)

    assert not b3.any() and not bs3.any(), \
        "kernel fast path folds the (zero) b3/bs3 into the DVE multiply"
    xt = x.reshape(T, DIM)
    top_i, top_w = _gate_host(xt, gate_w, gate_b)

    # Dispatch: token lists + gate weights per expert.  Capacity factor
    # 1.0 (C2 = T*TOPK/E = 2048): over-capacity pairs (lowest gate
    # weight first) are spilled to an exact host computation.
    NB, BS = 4, 512
    C2 = NB * BS
    idx, wgt, spills = [], [], []
    for e in range(E):
        toks = np.nonzero((top_i == e).any(axis=1))[0]
        slot = (top_i[toks] == e)            # [n_e, TOPK], one True per row
        we = top_w[toks][slot]
        if len(toks) > C2:
            order = np.argsort(-we, kind="stable")
            keep = np.sort(order[:C2])
            drop = np.sort(order[C2:])
            spills.append((e, toks[drop], we[drop]))
            toks, we = toks[keep], we[keep]
        idx.append(toks)
        wgt.append(we)

    ND, NP, NS = DIM // 128, DIM // 256, SHARED_INTER // 128

    def pack_w(wt, rows):
        # [rows, cols] -> [128, NP, 2, cols] with partition-contiguous rows
        return np.ascontiguousarray(
            wt.reshape(rows // 256, 2, 128, wt.shape[1]).transpose(2, 0, 1, 3))

    def pack_x(xt_, k):
        # [rows, cols] -> [128, k, cols]
        return np.ascontiguousarray(
            xt_.reshape(k, 128, xt_.shape[1]).transpose(1, 0, 2))

    ws1t = pack_x(ws1.T.astype(f16), ND)
    ws3t = pack_x(ws3.T.astype(f16), ND)
    ws2t = pack_x(ws2.T.astype(f16), NS)

    in_maps = []
    for e in range(E):
        xe = np.zeros((C2, DIM), np.float32)
        xe[:len(idx[e])] = xt[idx[e]]
        # [C2, DIM] -> [128, NB, ND, BS]
        xe8 = np.ascontiguousarray(
            xe.reshape(NB, BS, ND, 128).transpose(3, 0, 2, 1)).astype(f8)
        sl = slice(TS * e, TS * (e + 1))
        in_maps.append({
            "xe8": xe8,
            "xs": pack_x(np.ascontiguousarray(xt[sl].T).astype(f16), ND),
            "w1t8": pack_w((w1[e] * SW).T.astype(f8), DIM),
            "w3t8": pack_w((w3[e] * SW).T.astype(f8), DIM),
            "w2t8": pack_w((w2[e] * SW).T.astype(f8), INTER),
            "ws1t": ws1t, "ws3t": ws3t, "ws2t": ws2t,
            "biases": _pack_biases(b1[e], b3[e], b2[e], bs1, bs3, bs2),
        })

    key = (C2, BS, NB)
    if key not in _program_cache:
        _program_cache[key] = build_program(C2, BS, NB)
    nc = _program_cache[key]

    res = bass_utils.run_bass_kernel_spmd(
        nc, in_maps, core_ids=list(range(N_CORES)), trace=trace)

    y = np.empty((T, DIM), np.float32)
    for e in range(E):
        sl = slice(TS * e, TS * (e + 1))
        y[sl] = res.results[e]["ys"].T.astype(np.float32)
    for e in range(E):
        yef = res.results[e]["ye"].astype(np.float32)
        y[idx[e]] += yef[:, :len(idx[e])].T * wgt[e][:, None]
    # Spilled over-capacity pairs: exact fp32 on the host.
    for e, toks, we in spills:
        xs_ = xt[toks]
        h = _silu(xs_ @ w1[e].T + b1[e]) * (xs_ @ w3[e].T + b3[e])
        y[toks] += (h @ w2[e].T + b2[e]) * we[:, None]
    return y.reshape(B, S, DIM), res


def kernel(**inputs) -> np.ndarray:
    out, _ = run(inputs, trace=False)
    return out


# revision 8
# speedup vs baseline: 1.0489x; 1.0489x over previous
"""MoE (DeepSeek-style) routed+shared expert forward on 8 TRN2 NeuronCores.

Strategy (expert-parallel, host-side dispatch):
  - Host computes the gate (softmax + top-2) in float64 and gathers each
    expert's routed tokens (padded to a uniform capacity C2 = 2048,
    capacity factor 1.0; the handful of over-capacity pairs are computed
    exactly on the host and scattered in with the rest).
  - Core e runs expert e's routed tokens through the SwiGLU FFN in
    fp8(e4m3) with DoubleRow matmuls (2 k-tiles per instruction), plus a
    1/8 slice of all tokens through the replicated shared-expert MLP in
    fp16.
  - Weights are scaled by SW into e4m3; the activation instructions
    de-scale via their `scale` operand. h is stored fp8 so the w2 matmul
    also runs in DoubleRow mode.
  - One dma_start lands on one of the 16 DMA engines (~45 GB/s each), so
    inputs are split into ~128-512 KB single-writer chunk tiles issued in
    exact consumption order.  The critical first tiles alternate between
    the two HW-DGE trigger rings; everything else streams on the sync
    ring only, so the scalar sequencer dispatches the SILUs (which drain
    PSUM) without queueing behind descriptor writes.
  - Host scatters expert outputs back by routing index, scales by the
    gate weights, and adds the shared-expert output.
"""

import sys

if "/opt/trn_rl_repo" not in sys.path:
    sys.path.insert(0, "/opt/trn_rl_repo")

import ml_dtypes
import numpy as np

import concourse.bass as bass
import concourse.tile as tile
from concourse import bacc, mybir
from concourse import bass_utils

B, S, DIM = 4, 2048, 1024
T = B * S
INTER = 1024
E = 8
TOPK = 2
ROUTE_SCALE = 1.0
SHARED_INTER = 2048
N_CORES = 8
TS = T // N_CORES   # shared-expert tokens per core
SW = 16.0           # weight scale into e4m3; h8 = silu(z1)*(SW*z3) stays
                    # under e4m3 max 240 (measured |16h| <= ~102)

F32 = mybir.dt.float32
F16 = mybir.dt.float16
F8 = mybir.dt.float8e4
SILU = mybir.ActivationFunctionType.Silu
IDENT = mybir.ActivationFunctionType.Identity
DR = mybir.MatmulPerfMode.DoubleRow

_program_cache = {}


def build_program(C2, BS, NB):
    """Per-core SPMD Bass program. C2 = NB*BS routed capacity."""
    assert 2 <= NB <= 8, f"dr_chain streams NB blocks over an 8-bank PSUM ring, {NB=}"
    nc = bacc.Bacc("TRN2", target_bir_lowering=False, debug=False,
                   num_devices=N_CORES)

    def din(name, shape, dt):
        return nc.dram_tensor(name, shape, dt, kind="ExternalInput").ap()

    def dout(name, shape, dt):
        return nc.dram_tensor(name, shape, dt, kind="ExternalOutput").ap()

    ND = DIM // 128           # 8 k-tiles over DIM
    NI = INTER // 128
    NS = SHARED_INTER // 128  # 16
    NP = ND // 2              # k-tile pairs for DoubleRow
    H1 = INTER // 2

    # All inputs are host-packed in SBUF layout (partition dim first,
    # per-partition data contiguous) so each DMA is 128 large contiguous
    # descriptors instead of thousands of sub-KB ones.
    xe_r = din("xe8", (128, NB, ND, BS), F8)   # routed tokens
    w1_r = din("w1t8", (128, NP, 2, INTER), F8)
    w3_r = din("w3t8", (128, NP, 2, INTER), F8)
    w2_r = din("w2t8", (128, NP, 2, DIM), F8)
    xs_r = din("xs", (128, ND, TS), F16)       # shared-token slice
    ws1_r = din("ws1t", (128, ND, SHARED_INTER), F16)
    ws3_r = din("ws3t", (128, ND, SHARED_INTER), F16)
    ws2_r = din("ws2t", (128, NS, DIM), F16)
    biases = din("biases", (128, 64), F32)     # host-packed per-partition
    ye = dout("ye", (DIM, C2), F16)
    ys = dout("ys", (DIM, TS), F16)

    ye_r = ye.rearrange("(md p) c -> p md c", p=128)
    ys_r = ys.rearrange("(md p) c -> p md c", p=128)

    with tile.TileContext(nc) as tc:
        from contextlib import ExitStack
        es1 = ExitStack()
        with tc.tile_pool(name="bias", bufs=1) as bpool, \
             tc.tile_pool(name="wsh", bufs=1, side="right") as wspool, \
             tc.tile_pool(name="tmp", bufs=NB + 2) as tpool, \
             tc.tile_pool(name="yout", bufs=NB + 2) as ypool, \
             tc.tile_pool(name="ps", bufs=8, space="PSUM") as pspool:

            wpool = es1.enter_context(tc.tile_pool(name="wexp", bufs=1))
            xpool = es1.enter_context(tc.tile_pool(name="xep", bufs=1))
            hpool = es1.enter_context(tc.tile_pool(name="h8p", bufs=1))

            # ---- PE pre-warm: the HAM clock gate holds the PE at 1.2 GHz
            # until ~3.4us of sustained activity.  Run dummy matmuls on a
            # memset tile while the first real inputs stream in. ----
            warm = bpool.tile([128, 640], F8, tag="warm")
            nc.vector.memset(warm[:], 0)
            tw = bpool.tile([128, 16], F16, tag="tw")
            for i in range(10):
                pw = pspool.tile([128, 512], F32, tag="ps", name="ps",
                                 padded_shape=[128, 512])
                n = 512 if i < 8 else 128
                nc.tensor.matmul(pw[:, 0:n], warm[:, 0:128],
                                 warm[:, 128:128 + n],
                                 start=True, stop=True)

            ball = bpool.tile([128, 64], F32, tag="biases")
            b1_sb = ball[:, 0:NI]
            b2_sb = ball[:, 2 * NI:2 * NI + ND]
            bs1_sb = ball[:, 24:24 + NS]
            bs2_sb = ball[:, 24 + 2 * NS:24 + 2 * NS + ND]

            rings = [nc.sync, nc.scalar]
            ring_i = [0]

            def dma(dst, srcap):
                rings[ring_i[0] % 2].dma_start(dst, srcap)
                ring_i[0] += 1

            # ---- single-writer chunk tiles (one dma_start per tile) ----
            # w1/w3: per (j, colhalf): [128, 2, 512];  lhsT for (mi, j) is
            # w[j][mi//4][:, :, (mi%4)*128:...].
            w1_sb = [[wpool.tile([128, 2, H1], F8, tag=f"w1_{j}_{c}",
                                 name=f"w1_{j}_{c}") for c in range(2)]
                     for j in range(NP)]
            w3_sb = [[wpool.tile([128, 2, H1], F8, tag=f"w3_{j}_{c}",
                                 name=f"w3_{j}_{c}") for c in range(2)]
                     for j in range(NP)]
            w2_sb = [[wpool.tile([128, 2, H1], F8, tag=f"w2_{j}_{c}",
                                 name=f"w2_{j}_{c}") for c in range(2)]
                     for j in range(NP)]
            # xe: per (b, j): [128, 2, BS] — exactly one DoubleRow rhs.
            xe_sb = [[xpool.tile([128, 2, BS], F8, tag=f"xe{b}_{j}",
                                 name=f"xe{b}_{j}") for j in range(NP)]
                     for b in range(NB)]

            def w_of(w, mi, j):
                return w[j][mi // 4][:, :, (mi % 4) * 128:(mi % 4 + 1) * 128]

            # Critical preamble, in consumption order, alternating rings.
            # Scalar ring gets the biases + a bounded number of chunks so
            # the first SILU dispatches before PSUM fills (~8 bank-pairs).
            nc.scalar.dma_start(ball[:], biases[:])
            for j in range(NP):
                dma(w1_sb[j][0][:], w1_r[:, j, :, 0:H1])
            for j in range(NP):
                dma(xe_sb[0][j][:], xe_r[:, 0, 2 * j:2 * j + 2, :])
            # ACT tables: preload on the scalar engine before first SILU.
            nc.scalar.activation(tw[:], warm[:, 0:16], SILU)
            nc.scalar.activation(tw[:], warm[:, 0:16], IDENT)
            for j in range(NP):
                dma(w3_sb[j][0][:], w3_r[:, j, :, 0:H1])
            for j in range(NP):
                dma(xe_sb[1][j][:], xe_r[:, 1, 2 * j:2 * j + 2, :])

            # Everything else is split across both ring-engine groups:
            # even chunks issue immediately on the sync ring (whose
            # sequencer is otherwise idle in phase-1 layer-1); odd chunks
            # are paced onto the scalar ring between SILU dispatches so
            # the PSUM-draining SILUs never queue behind descriptor
            # writes.
            from collections import deque
            paced = deque()

            def split_load(dst, src, which):
                if which == 0:
                    nc.sync.dma_start(dst, src)
                else:
                    paced.append((dst, src))

            def drain(n):
                for _ in range(min(n, len(paced))):
                    dst, src = paced.popleft()
                    nc.scalar.dma_start(dst, src)

            for b in range(2, NB):
                for j in range(NP):
                    nc.sync.dma_start(xe_sb[b][j][:],
                                      xe_r[:, b, 2 * j:2 * j + 2, :])
            xs_sb = [wspool.tile([128, 2, TS], F16, tag=f"xs{q}",
                                 name=f"xs{q}") for q in range(ND // 2)]
            ws1_sb = [wspool.tile([128, 1, SHARED_INTER], F16, tag=f"ws1_{k}",
                                  name=f"ws1_{k}") for k in range(ND)]
            ws3_sb = [wspool.tile([128, 1, SHARED_INTER], F16, tag=f"ws3_{k}",
                                  name=f"ws3_{k}") for k in range(ND)]
            ws2_sb = [wspool.tile([128, 4, DIM], F16, tag=f"ws2_{q}",
                                  name=f"ws2_{q}") for q in range(NS // 4)]
            for j in range(NP):
                split_load(w1_sb[j][1][:], w1_r[:, j, :, H1:INTER], j % 2)
            for j in range(NP):
                split_load(w3_sb[j][1][:], w3_r[:, j, :, H1:INTER], j % 2)
            for c in range(2):
                for j in range(NP):
                    split_load(w2_sb[j][c][:],
                               w2_r[:, j, :, c * H1:(c + 1) * H1], j % 2)
            for q in range(ND // 2):
                split_load(xs_sb[q][:], xs_r[:, 2 * q:2 * q + 2, :], q % 2)
            for k in range(ND):
                split_load(ws1_sb[k][:], ws1_r[:, k:k + 1, :], k % 2)
            for k in range(ND):
                split_load(ws3_sb[k][:], ws3_r[:, k:k + 1, :], k % 2)
            for q in range(NS // 4):
                split_load(ws2_sb[q][:], ws2_r[:, 4 * q:4 * q + 4, :], q % 2)

            h_sb = [hpool.tile([128, NI, BS], F8, tag=f"h{b}", name=f"h{b}")
                    for b in range(NB)]

            # ================= Phase 1: routed expert (fp8 DoubleRow) ====
            for mi in range(NI):
                # Per-block z1 -> silu -> z3 -> mul: spreads the early
                # xe-block DMA demand twice as thin as z1-for-all-blocks
                # first, so the head streams without stalling the PE.
                for b in range(NB):
                    ps1 = pspool.tile([128, BS], F32, tag="ps", name="ps",
                                      padded_shape=[128, 512])
                    for j in range(NP):
                        nc.tensor.matmul(
                            ps1[:], w_of(w1_sb, mi, j), xe_sb[b][j][:],
                            start=(j == 0), stop=(j == NP - 1),
                            perf_mode=DR)
                    t1 = tpool.tile([128, BS], F16, tag="t1",
                                    padded_shape=[128, 512])
                    nc.scalar.activation(t1[:], ps1[:], SILU,
                                         bias=b1_sb[:, mi:mi + 1],
                                         scale=1.0 / SW)
                    ps3 = pspool.tile([128, BS], F32, tag="ps", name="ps",
                                      padded_shape=[128, 512])
                    for j in range(NP):
                        nc.tensor.matmul(
                            ps3[:], w_of(w3_sb, mi, j), xe_sb[b][j][:],
                            start=(j == 0), stop=(j == NP - 1),
                            perf_mode=DR)
                    # b3 is zero, so h8 = t1 * (SW*z3) reads PSUM directly
                    nc.vector.tensor_mul(h_sb[b][:, mi, :], t1[:], ps3[:])
                    if mi >= 1 and b % 2 == 0:
                        drain(2)

            for md in range(ND):
                # One weight load streams all NB blocks (j-outer).
                pss = [pspool.tile([128, BS], F32, tag="ps", name="ps",
                                   padded_shape=[128, 512])
                       for _ in range(NB)]
                for j in range(NP):
                    for b in range(NB):
                        nc.tensor.matmul(
                            pss[b][:], w_of(w2_sb, md, j),
                            h_sb[b][:, 2 * j:2 * j + 2, :],
                            start=(j == 0), stop=(j == NP - 1),
                            perf_mode=DR)
                for b in range(NB):
                    yt = ypool.tile([128, BS], F16, tag="yt",
                                    padded_shape=[128, 512])
                    nc.scalar.activation(yt[:], pss[b][:], IDENT,
                                         bias=b2_sb[:, md:md + 1],
                                         scale=1.0 / (SW * SW))
                    dma(ye_r[:, md, b * BS:(b + 1) * BS], yt[:])
                drain(2)

            es1.close()  # free phase-1 pools; hsp reuses their space

            # ================= Phase 2: shared expert (fp16) =============
            NBS = 2
            BSS = TS // NBS  # 512
            hspool = ExitStack()
            hsp = hspool.enter_context(tc.tile_pool(name="hsp", bufs=1))
            hs_sb = [hsp.tile([128, NS, BSS], F16, tag=f"hs{b}", name=f"hs{b}")
                     for b in range(NBS)]

            def f16_chain(w_sb, msel, n):
                pss = [pspool.tile([128, n], F32, tag="ps", name="ps",
                                   padded_shape=[128, 512])
                       for _ in range(NBS)]
                for k in range(ND):
                    for b in range(NBS):
                        nc.tensor.matmul(
                            pss[b][:],
                            w_sb[k][:, 0, msel],
                            xs_sb[k // 2][:, k % 2, b * n:(b + 1) * n],
                            start=(k == 0), stop=(k == ND - 1))
                return pss

            for mi in range(NS):
                msel = slice(mi * 128, (mi + 1) * 128)
                ps1 = f16_chain(ws1_sb, msel, BSS)
                t1s = []
                for b in range(NBS):
                    t1 = tpool.tile([128, BSS], F16, tag="t1",
                                    padded_shape=[128, 512])
                    nc.scalar.activation(t1[:], ps1[b][:], SILU,
                                         bias=bs1_sb[:, mi:mi + 1])
                    t1s.append(t1)
                ps3 = f16_chain(ws3_sb, msel, BSS)
                for b in range(NBS):
                    nc.vector.tensor_mul(hs_sb[b][:, mi, :], t1s[b][:],
                                         ps3[b][:])

            for md in range(ND):
                msel = slice(md * 128, (md + 1) * 128)
                pss = [pspool.tile([128, BSS], F32, tag="ps", name="ps",
                                   padded_shape=[128, 512])
                       for _ in range(NBS)]
                for ji in range(NS):
                    for b in range(NBS):
                        nc.tensor.matmul(
                            pss[b][:],
                            ws2_sb[ji // 4][:, ji % 4, msel],
                            hs_sb[b][:, ji, :],
                            start=(ji == 0), stop=(ji == NS - 1))
                for b in range(NBS):
                    yt = ypool.tile([128, BSS], F16, tag="yt",
                                    padded_shape=[128, 512])
                    nc.scalar.activation(yt[:], pss[b][:], IDENT,
                                         bias=bs2_sb[:, md:md + 1])
                    # Split the store so the tail-critical final store
                    # lands on two engines instead of one.
                    hb = BSS // 2
                    dma(ys_r[:, md, b * BSS:b * BSS + hb], yt[:, 0:hb])
                    dma(ys_r[:, md, b * BSS + hb:(b + 1) * BSS], yt[:, hb:])
            hspool.close()

    nc.compile()
    return nc


def _pack_biases(b1, b3, b2, bs1, bs3, bs2):
    """Pack all bias vectors into one [128, 64] per-partition table."""
    out = np.zeros((128, 64), np.float32)
    cols = [(b1, 0), (b3, 8), (b2, 16), (bs1, 24), (bs3, 40),
            (bs2, 56)]
    for vec, c0 in cols:
        k = len(vec) // 128
        out[:, c0:c0 + k] = vec.reshape(k, 128).T
    return out


def _gate_host(xt, gate_w, gate_b):
    """Softmax gate + top-2 routing, computed in float64 on the host."""
    logits = xt.astype(np.float64) @ gate_w.astype(np.float64).T \
        + gate_b.astype(np.float64)
    m = logits.max(axis=-1, keepdims=True)
    p = np.exp(logits - m)
    scores = p / p.sum(axis=-1, keepdims=True)
    order = np.argsort(-scores, axis=1, kind="stable")
    top_i = order[:, :TOPK]
    top_w = (np.take_along_axis(scores, top_i, axis=1)
             * ROUTE_SCALE).astype(np.float32)
    return top_i, top_w


def _silu(z):
    return z / (1.0 + np.exp(-z))


def run(inputs, trace=False):
    f8 = ml_dtypes.float8_e4m3   # TRN-style e4m3 (max 240)
    f16 = np.float16

    x = np.ascontiguousarray(np.asarray(inputs["x"], dtype=np.float32))
    gate_w = np.asarray(inputs["gate_w"], dtype=np.float32)
    gate_b = np.asarray(inputs["gate_b"], dtype=np.float32)
    w1 = np.asarray(inputs["w1"], dtype=np.float32)
    b1 = np.asarray(inputs["b1"], dtype=np.float32)
    w3 = np.asarray(inputs["w3"], dtype=np.float32)
    b3 = np.asarray(inputs["b3"], dtype=np.float32)
    w2 = np.asarray(inputs["w2"], dtype=np.float32)
    b2 = np.asarray(inputs["b2"], dtype=np.float32)
    ws1 = np.asarray(inputs["ws1"], dtype=np.float32)
    bs1 = np.asarray(inputs["bs1"], dtype=np.float32)
    ws3 = np.asarray(inputs["ws3"], dtype=np.float32)
    bs3 = np.asarray(inputs["bs3"], dtype=np.float32)
    ws2 = np.asarray(inputs["ws2"], dtype=np.float32)
    bs2 = np.asarray(inputs["bs2"], dtype=np.float32)

    assert not b3.any() and not bs3.any(), \
        "kernel fast path folds the (zero) b3/bs3 into the DVE multiply"
    xt = x.reshape(T, DIM)
    top_i, top_w = _gate_host(xt, gate_w, gate_b)

    # Dispatch: token lists + gate weights per expert.  Capacity factor
    # 1.0 (C2 = T*TOPK/E = 2048): over-capacity pairs (lowest gate
    # weight first) are spilled to an exact host computation.
    NB, BS = 4, 512
    C2 = NB * BS
    idx, wgt, spills = [], [], []
    for e in range(E):
        toks = np.nonzero((top_i == e).any(axis=1))[0]
        slot = (top_i[toks] == e)            # [n_e, TOPK], one True per row
        we = top_w[toks][slot]
        if len(toks) > C2:
            order = np.argsort(-we, kind="stable")
            keep = np.sort(order[:C2])
            drop = np.sort(order[C2:])
            spills.append((e, toks[drop], we[drop]))
            toks, we = toks[keep], we[keep]
        idx.append(toks)
        wgt.append(we)

    ND, NP, NS = DIM // 128, DIM // 256, SHARED_INTER // 128

    def pack_w(wt, rows):
        # [rows, cols] -> [128, NP, 2, cols] with partition-contiguous rows
        return np.ascontiguousarray(
            wt.reshape(rows // 256, 2, 128, wt.shape[1]).transpose(2, 0, 1, 3))

    def pack_x(xt_, k):
        # [rows, cols] -> [128, k, cols]
        return np.ascontiguousarray(
            xt_.reshape(k, 128, xt_.shape[1]).transpose(1, 0, 2))

    ws1t = pack_x(ws1.T.astype(f16), ND)
    ws3t = pack_x(ws3.T.astype(f16), ND)
    ws2t = pack_x(ws2.T.astype(f16), NS)

    in_maps = []
    for e in range(E):
        xe = np.zeros((C2, DIM), np.float32)
        xe[:len(idx[e])] = xt[idx[e]]
        # [C2, DIM] -> [128, NB, ND, BS]
        xe8 = np.ascontiguousarray(
            xe.reshape(NB, BS, ND, 128).transpose(3, 0, 2, 1)).astype(f8)
        sl = slice(TS * e, TS * (e + 1))
        in_maps.append({
            "xe8": xe8,
            "xs": pack_x(np.ascontiguousarray(xt[sl].T).astype(f16), ND),
            "w1t8": pack_w((w1[e] * SW).T.astype(f8), DIM),
            "w3t8": pack_w((w3[e] * SW).T.astype(f8), DIM),
            "w2t8": pack_w((w2[e] * SW).T.astype(f8), INTER),
            "ws1t": ws1t, "ws3t": ws3t, "ws2t": ws2t,
            "biases": _pack_biases(b1[e], b3[e], b2[e], bs1, bs3, bs2),
        })

    key = (C2, BS, NB)
    if key not in _program_cache:
        _program_cache[key] = build_program(C2, BS, NB)
    nc = _program_cache[key]

    res = bass_utils.run_bass_kernel_spmd(
        nc, in_maps, core_ids=list(range(N_CORES)), trace=trace)

    y = np.empty((T, DIM), np.float32)
    for e in range(E):
        sl = slice(TS * e, TS * (e + 1))
        y[sl] = res.results[e]["ys"].T.astype(np.float32)
    for e in range(E):
        yef = res.results[e]["ye"].astype(np.float32)
        y[idx[e]] += yef[:, :len(idx[e])].T * wgt[e][:, None]
    # Spilled over-capacity pairs: exact fp32 on the host.
    for e, toks, we in spills:
        xs_ = xt[toks]
        h = _silu(xs_ @ w1[e].T + b1[e]) * (xs_ @ w3[e].T + b3[e])
        y[toks] += (h @ w2[e].T + b2[e]) * we[:, None]
    return y.reshape(B, S, DIM), res


def kernel(**inputs) -> np.ndarray:
    out, _ = run(inputs, trace=False)
    return out


# revision 10
# speedup vs baseline: 1.0511x; 1.0021x over previous
"""MoE (DeepSeek-style) routed+shared expert forward on 8 TRN2 NeuronCores.

Strategy (expert-parallel, host-side dispatch):
  - Host computes the gate (softmax + top-2) in float64 and gathers each
    expert's routed tokens (padded to a uniform capacity C2 = 2048,
    capacity factor 1.0; the handful of over-capacity pairs are computed
    exactly on the host and scattered in with the rest).
  - Core e runs expert e's routed tokens through the SwiGLU FFN in
    fp8(e4m3) with DoubleRow matmuls (2 k-tiles per instruction), plus a
    1/8 slice of all tokens through the replicated shared-expert MLP in
    fp16.
  - Weights are scaled by SW into e4m3; the activation instructions
    de-scale via their `scale` operand. h is stored fp8 so the w2 matmul
    also runs in DoubleRow mode.
  - One dma_start lands on one of the 16 DMA engines (~45 GB/s each), so
    inputs are split into ~128-512 KB single-writer chunk tiles issued in
    exact consumption order.  The critical first tiles alternate between
    the two HW-DGE trigger rings; everything else streams on the sync
    ring only, so the scalar sequencer dispatches the SILUs (which drain
    PSUM) without queueing behind descriptor writes.
  - Host scatters expert outputs back by routing index, scales by the
    gate weights, and adds the shared-expert output.
"""

import sys

if "/opt/trn_rl_repo" not in sys.path:
    sys.path.insert(0, "/opt/trn_rl_repo")

import ml_dtypes
import numpy as np

import concourse.bass as bass
import concourse.tile as tile
from concourse import bacc, mybir
from concourse import bass_utils

B, S, DIM = 4, 2048, 1024
T = B * S
INTER = 1024
E = 8
TOPK = 2
ROUTE_SCALE = 1.0
SHARED_INTER = 2048
N_CORES = 8
TS = T // N_CORES   # shared-expert tokens per core
SW = 16.0           # weight scale into e4m3; h8 = silu(z1)*(SW*z3) stays
                    # under e4m3 max 240 (measured |16h| <= ~102)

F32 = mybir.dt.float32
F16 = mybir.dt.float16
F8 = mybir.dt.float8e4
SILU = mybir.ActivationFunctionType.Silu
IDENT = mybir.ActivationFunctionType.Identity
DR = mybir.MatmulPerfMode.DoubleRow

_program_cache = {}


def build_program(C2, BS, NB):
    """Per-core SPMD Bass program. C2 = NB*BS routed capacity."""
    assert 2 <= NB <= 8, f"dr_chain streams NB blocks over an 8-bank PSUM ring, {NB=}"
    nc = bacc.Bacc("TRN2", target_bir_lowering=False, debug=False,
                   num_devices=N_CORES)

    def din(name, shape, dt):
        return nc.dram_tensor(name, shape, dt, kind="ExternalInput").ap()

    def dout(name, shape, dt):
        return nc.dram_tensor(name, shape, dt, kind="ExternalOutput").ap()

    ND = DIM // 128           # 8 k-tiles over DIM
    NI = INTER // 128
    NS = SHARED_INTER // 128  # 16
    NP = ND // 2              # k-tile pairs for DoubleRow
    H1 = INTER // 2

    # All inputs are host-packed in SBUF layout (partition dim first,
    # per-partition data contiguous) so each DMA is 128 large contiguous
    # descriptors instead of thousands of sub-KB ones.
    xe_r = din("xe8", (128, NB, ND, BS), F8)   # routed tokens
    w1_r = din("w1t8", (128, NP, 2, INTER), F8)
    w3_r = din("w3t8", (128, NP, 2, INTER), F8)
    w2_r = din("w2t8", (128, NP, 2, DIM), F8)
    xs_r = din("xs", (128, ND, TS), F16)       # shared-token slice
    ws1_r = din("ws1t", (128, ND, SHARED_INTER), F16)
    ws3_r = din("ws3t", (128, ND, SHARED_INTER), F16)
    ws2_r = din("ws2t", (128, NS, DIM), F16)
    biases = din("biases", (128, 64), F32)     # host-packed per-partition
    ye = dout("ye", (DIM, C2), F16)
    ys = dout("ys", (DIM, TS), F16)

    ye_r = ye.rearrange("(md p) c -> p md c", p=128)
    ys_r = ys.rearrange("(md p) c -> p md c", p=128)

    with tile.TileContext(nc) as tc:
        from contextlib import ExitStack
        es1 = ExitStack()
        with tc.tile_pool(name="bias", bufs=1) as bpool, \
             tc.tile_pool(name="wsh", bufs=1, side="right") as wspool, \
             tc.tile_pool(name="tmp", bufs=NB + 2) as tpool, \
             tc.tile_pool(name="yout", bufs=NB + 2) as ypool, \
             tc.tile_pool(name="ps", bufs=8, space="PSUM") as pspool:

            wpool = es1.enter_context(tc.tile_pool(name="wexp", bufs=1))
            xpool = es1.enter_context(tc.tile_pool(name="xep", bufs=1))
            hpool = es1.enter_context(tc.tile_pool(name="h8p", bufs=1))

            # ---- PE pre-warm: the HAM clock gate holds the PE at 1.2 GHz
            # until ~3.4us of sustained activity.  Run dummy matmuls on a
            # memset tile while the first real inputs stream in. ----
            warm = bpool.tile([128, 640], F8, tag="warm")
            nc.vector.memset(warm[:], 0)
            tw = bpool.tile([128, 16], F16, tag="tw")
            for i in range(10):
                pw = pspool.tile([128, 512], F32, tag="ps", name="ps",
                                 padded_shape=[128, 512])
                n = 512 if i < 8 else 128
                nc.tensor.matmul(pw[:, 0:n], warm[:, 0:128],
                                 warm[:, 128:128 + n],
                                 start=True, stop=True)

            ball = bpool.tile([128, 64], F32, tag="biases")
            b1_sb = ball[:, 0:NI]
            b2_sb = ball[:, 2 * NI:2 * NI + ND]
            bs1_sb = ball[:, 24:24 + NS]
            bs2_sb = ball[:, 24 + 2 * NS:24 + 2 * NS + ND]

            rings = [nc.sync, nc.scalar]
            ring_i = [0]

            def dma(dst, srcap):
                rings[ring_i[0] % 2].dma_start(dst, srcap)
                ring_i[0] += 1

            # ---- single-writer chunk tiles (one dma_start per tile) ----
            # w1/w3: per (j, colhalf): [128, 2, 512];  lhsT for (mi, j) is
            # w[j][mi//4][:, :, (mi%4)*128:...].
            w1_sb = [[wpool.tile([128, 2, H1], F8, tag=f"w1_{j}_{c}",
                                 name=f"w1_{j}_{c}") for c in range(2)]
                     for j in range(NP)]
            w3_sb = [[wpool.tile([128, 2, H1], F8, tag=f"w3_{j}_{c}",
                                 name=f"w3_{j}_{c}") for c in range(2)]
                     for j in range(NP)]
            w2_sb = [[wpool.tile([128, 2, H1], F8, tag=f"w2_{j}_{c}",
                                 name=f"w2_{j}_{c}") for c in range(2)]
                     for j in range(NP)]
            # xe: per (b, j): [128, 2, BS] — exactly one DoubleRow rhs.
            xe_sb = [[xpool.tile([128, 2, BS], F8, tag=f"xe{b}_{j}",
                                 name=f"xe{b}_{j}") for j in range(NP)]
                     for b in range(NB)]

            def w_of(w, mi, j):
                return w[j][mi // 4][:, :, (mi % 4) * 128:(mi % 4 + 1) * 128]

            # Critical preamble, in consumption order, alternating rings.
            # Scalar ring gets the biases + a bounded number of chunks so
            # the first SILU dispatches before PSUM fills (~8 bank-pairs).
            nc.scalar.dma_start(ball[:], biases[:])
            # Interleave w1 k-pairs with the xe block they multiply so the
            # first chain's operands arrive in chain order.
            for j in range(NP):
                dma(w1_sb[j][0][:], w1_r[:, j, :, 0:H1])
                dma(xe_sb[0][j][:], xe_r[:, 0, 2 * j:2 * j + 2, :])
            # ACT tables: preload on the scalar engine before first SILU.
            nc.scalar.activation(tw[:], warm[:, 0:16], SILU)
            nc.scalar.activation(tw[:], warm[:, 0:16], IDENT)
            for j in range(NP):
                dma(w3_sb[j][0][:], w3_r[:, j, :, 0:H1])
            for j in range(NP):
                dma(xe_sb[1][j][:], xe_r[:, 1, 2 * j:2 * j + 2, :])

            # Everything else is split across both ring-engine groups:
            # even chunks issue immediately on the sync ring (whose
            # sequencer is otherwise idle in phase-1 layer-1); odd chunks
            # are paced onto the scalar ring between SILU dispatches so
            # the PSUM-draining SILUs never queue behind descriptor
            # writes.
            from collections import deque
            paced = deque()

            def split_load(dst, src, which):
                if which == 0:
                    nc.sync.dma_start(dst, src)
                else:
                    paced.append((dst, src))

            def drain(n):
                for _ in range(min(n, len(paced))):
                    dst, src = paced.popleft()
                    nc.scalar.dma_start(dst, src)

            for b in range(2, NB):
                for j in range(NP):
                    nc.sync.dma_start(xe_sb[b][j][:],
                                      xe_r[:, b, 2 * j:2 * j + 2, :])
            xs_sb = [wspool.tile([128, 2, TS], F16, tag=f"xs{q}",
                                 name=f"xs{q}") for q in range(ND // 2)]
            ws1_sb = [wspool.tile([128, 1, SHARED_INTER], F16, tag=f"ws1_{k}",
                                  name=f"ws1_{k}") for k in range(ND)]
            ws3_sb = [wspool.tile([128, 1, SHARED_INTER], F16, tag=f"ws3_{k}",
                                  name=f"ws3_{k}") for k in range(ND)]
            ws2_sb = [wspool.tile([128, 4, DIM], F16, tag=f"ws2_{q}",
                                  name=f"ws2_{q}") for q in range(NS // 4)]
            for j in range(NP):
                split_load(w1_sb[j][1][:], w1_r[:, j, :, H1:INTER], j % 2)
            for j in range(NP):
                split_load(w3_sb[j][1][:], w3_r[:, j, :, H1:INTER], j % 2)
            for c in range(2):
                for j in range(NP):
                    split_load(w2_sb[j][c][:],
                               w2_r[:, j, :, c * H1:(c + 1) * H1], j % 2)
            for q in range(ND // 2):
                split_load(xs_sb[q][:], xs_r[:, 2 * q:2 * q + 2, :], q % 2)
            for k in range(ND):
                split_load(ws1_sb[k][:], ws1_r[:, k:k + 1, :], k % 2)
            for k in range(ND):
                split_load(ws3_sb[k][:], ws3_r[:, k:k + 1, :], k % 2)
            for q in range(NS // 4):
                split_load(ws2_sb[q][:], ws2_r[:, 4 * q:4 * q + 4, :], q % 2)

            h_sb = [hpool.tile([128, NI, BS], F8, tag=f"h{b}", name=f"h{b}")
                    for b in range(NB)]

            # ================= Phase 1: routed expert (fp8 DoubleRow) ====
            for mi in range(NI):
                # Per-block z1 -> silu -> z3 -> mul: spreads the early
                # xe-block DMA demand twice as thin as z1-for-all-blocks
                # first, so the head streams without stalling the PE.
                for b in range(NB):
                    ps1 = pspool.tile([128, BS], F32, tag="ps", name="ps",
                                      padded_shape=[128, 512])
                    for j in range(NP):
                        nc.tensor.matmul(
                            ps1[:], w_of(w1_sb, mi, j), xe_sb[b][j][:],
                            start=(j == 0), stop=(j == NP - 1),
                            perf_mode=DR)
                    t1 = tpool.tile([128, BS], F16, tag="t1",
                                    padded_shape=[128, 512])
                    nc.scalar.activation(t1[:], ps1[:], SILU,
                                         bias=b1_sb[:, mi:mi + 1],
                                         scale=1.0 / SW)
                    ps3 = pspool.tile([128, BS], F32, tag="ps", name="ps",
                                      padded_shape=[128, 512])
                    for j in range(NP):
                        nc.tensor.matmul(
                            ps3[:], w_of(w3_sb, mi, j), xe_sb[b][j][:],
                            start=(j == 0), stop=(j == NP - 1),
                            perf_mode=DR)
                    # b3 is zero, so h8 = t1 * (SW*z3) reads PSUM directly
                    nc.vector.tensor_mul(h_sb[b][:, mi, :], t1[:], ps3[:])
                    if mi >= 1 and b % 2 == 0:
                        drain(2)

            for md in range(ND):
                # One weight load streams all NB blocks (j-outer).
                pss = [pspool.tile([128, BS], F32, tag="ps", name="ps",
                                   padded_shape=[128, 512])
                       for _ in range(NB)]
                for j in range(NP):
                    for b in range(NB):
                        nc.tensor.matmul(
                            pss[b][:], w_of(w2_sb, md, j),
                            h_sb[b][:, 2 * j:2 * j + 2, :],
                            start=(j == 0), stop=(j == NP - 1),
                            perf_mode=DR)
                for b in range(NB):
                    yt = ypool.tile([128, BS], F16, tag="yt",
                                    padded_shape=[128, 512])
                    nc.scalar.activation(yt[:], pss[b][:], IDENT,
                                         bias=b2_sb[:, md:md + 1],
                                         scale=1.0 / (SW * SW))
                    dma(ye_r[:, md, b * BS:(b + 1) * BS], yt[:])
                drain(2)

            es1.close()  # free phase-1 pools; hsp reuses their space

            # ================= Phase 2: shared expert (fp16) =============
            NBS = 2
            BSS = TS // NBS  # 512
            hspool = ExitStack()
            hsp = hspool.enter_context(tc.tile_pool(name="hsp", bufs=1))
            hs_sb = [hsp.tile([128, NS, BSS], F16, tag=f"hs{b}", name=f"hs{b}")
                     for b in range(NBS)]

            def f16_chain(w_sb, msel, n):
                pss = [pspool.tile([128, n], F32, tag="ps", name="ps",
                                   padded_shape=[128, 512])
                       for _ in range(NBS)]
                for k in range(ND):
                    for b in range(NBS):
                        nc.tensor.matmul(
                            pss[b][:],
                            w_sb[k][:, 0, msel],
                            xs_sb[k // 2][:, k % 2, b * n:(b + 1) * n],
                            start=(k == 0), stop=(k == ND - 1))
                return pss

            for mi in range(NS):
                msel = slice(mi * 128, (mi + 1) * 128)
                ps1 = f16_chain(ws1_sb, msel, BSS)
                t1s = []
                for b in range(NBS):
                    t1 = tpool.tile([128, BSS], F16, tag="t1",
                                    padded_shape=[128, 512])
                    nc.scalar.activation(t1[:], ps1[b][:], SILU,
                                         bias=bs1_sb[:, mi:mi + 1])
                    t1s.append(t1)
                ps3 = f16_chain(ws3_sb, msel, BSS)
                for b in range(NBS):
                    nc.vector.tensor_mul(hs_sb[b][:, mi, :], t1s[b][:],
                                         ps3[b][:])

            for md in range(ND):
                msel = slice(md * 128, (md + 1) * 128)
                pss = [pspool.tile([128, BSS], F32, tag="ps", name="ps",
                                   padded_shape=[128, 512])
                       for _ in range(NBS)]
                for ji in range(NS):
                    for b in range(NBS):
                        nc.tensor.matmul(
                            pss[b][:],
                            ws2_sb[ji // 4][:, ji % 4, msel],
                            hs_sb[b][:, ji, :],
                            start=(ji == 0), stop=(ji == NS - 1))
                if md < ND - 1:
                    for b in range(NBS):
                        yt = ypool.tile([128, BSS], F16, tag="yt",
                                        padded_shape=[128, 512])
                        nc.scalar.activation(yt[:], pss[b][:], IDENT,
                                             bias=bs2_sb[:, md:md + 1])
                        # Split the store across two engines.
                        hb = BSS // 2
                        dma(ys_r[:, md, b * BSS:b * BSS + hb], yt[:, 0:hb])
                        dma(ys_r[:, md, b * BSS + hb:(b + 1) * BSS],
                            yt[:, hb:])
                else:
                    # Last output block is tail-critical: emit narrow
                    # IDENT+store sub-chunks so the final store after the
                    # last matmul is small (asymmetric 384+128 split).
                    for b in range(NBS):
                        for lo, hi in [(0, 384), (384, BSS)]:
                            yt = ypool.tile([128, hi - lo], F16, tag="yt",
                                            padded_shape=[128, 512])
                            nc.scalar.activation(yt[:], pss[b][:, lo:hi],
                                                 IDENT,
                                                 bias=bs2_sb[:, md:md + 1])
                            dma(ys_r[:, md, b * BSS + lo:b * BSS + hi],
                                yt[:])
            hspool.close()

    nc.compile()
    return nc


def _pack_biases(b1, b3, b2, bs1, bs3, bs2):
    """Pack all bias vectors into one [128, 64] per-partition table."""
    out = np.zeros((128, 64), np.float32)
    cols = [(b1, 0), (b3, 8), (b2, 16), (bs1, 24), (bs3, 40),
            (bs2, 56)]
    for vec, c0 in cols:
        k = len(vec) // 128
        out[:, c0:c0 + k] = vec.reshape(k, 128).T
    return out


def _gate_host(xt, gate_w, gate_b):
    """Softmax gate + top-2 routing, computed in float64 on the host."""
    logits = xt.astype(np.float64) @ gate_w.astype(np.float64).T \
        + gate_b.astype(np.float64)
    m = logits.max(axis=-1, keepdims=True)
    p = np.exp(logits - m)
    scores = p / p.sum(axis=-1, keepdims=True)
    order = np.argsort(-scores, axis=1, kind="stable")
    top_i = order[:, :TOPK]
    top_w = (np.take_along_axis(scores, top_i, axis=1)
             * ROUTE_SCALE).astype(np.float32)
    return top_i, top_w


def _silu(z):
    return z / (1.0 + np.exp(-z))


def run(inputs, trace=False):
    f8 = ml_dtypes.float8_e4m3   # TRN-style e4m3 (max 240)
    f16 = np.float16

    x = np.ascontiguousarray(np.asarray(inputs["x"], dtype=np.float32))
    gate_w = np.asarray(inputs["gate_w"], dtype=np.float32)
    gate_b = np.asarray(inputs["gate_b"], dtype=np.float32)
    w1 = np.asarray(inputs["w1"], dtype=np.float32)
    b1 = np.asarray(inputs["b1"], dtype=np.float32)
    w3 = np.asarray(inputs["w3"], dtype=np.float32)
    b3 = np.asarray(inputs["b3"], dtype=np.float32)
    w2 = np.asarray(inputs["w2"], dtype=np.float32)
    b2 = np.asarray(inputs["b2"], dtype=np.float32)
    ws1 = np.asarray(inputs["ws1"], dtype=np.float32)
    bs1 = np.asarray(inputs["bs1"], dtype=np.float32)
    ws3 = np.asarray(inputs["ws3"], dtype=np.float32)
    bs3 = np.asarray(inputs["bs3"], dtype=np.float32)
    ws2 = np.asarray(inputs["ws2"], dtype=np.float32)
    bs2 = np.asarray(inputs["bs2"], dtype=np.float32)

    assert not b3.any() and not bs3.any(), \
        "kernel fast path folds the (zero) b3/bs3 into the DVE multiply"
    xt = x.reshape(T, DIM)
    top_i, top_w = _gate_host(xt, gate_w, gate_b)

    # Dispatch: token lists + gate weights per expert.  Capacity factor
    # 1.0 (C2 = T*TOPK/E = 2048): over-capacity pairs (lowest gate
    # weight first) are spilled to an exact host computation.
    NB, BS = 4, 512
    C2 = NB * BS
    idx, wgt, spills = [], [], []
    for e in range(E):
        toks = np.nonzero((top_i == e).any(axis=1))[0]
        slot = (top_i[toks] == e)            # [n_e, TOPK], one True per row
        we = top_w[toks][slot]
        if len(toks) > C2:
            order = np.argsort(-we, kind="stable")
            keep = np.sort(order[:C2])
            drop = np.sort(order[C2:])
            spills.append((e, toks[drop], we[drop]))
            toks, we = toks[keep], we[keep]
        idx.append(toks)
        wgt.append(we)

    ND, NP, NS = DIM // 128, DIM // 256, SHARED_INTER // 128

    def pack_w(wt, rows):
        # [rows, cols] -> [128, NP, 2, cols] with partition-contiguous rows
        return np.ascontiguousarray(
            wt.reshape(rows // 256, 2, 128, wt.shape[1]).transpose(2, 0, 1, 3))

    def pack_x(xt_, k):
        # [rows, cols] -> [128, k, cols]
        return np.ascontiguousarray(
            xt_.reshape(k, 128, xt_.shape[1]).transpose(1, 0, 2))

    ws1t = pack_x(ws1.T.astype(f16), ND)
    ws3t = pack_x(ws3.T.astype(f16), ND)
    ws2t = pack_x(ws2.T.astype(f16), NS)

    in_maps = []
    for e in range(E):
        xe = np.zeros((C2, DIM), np.float32)
        xe[:len(idx[e])] = xt[idx[e]]
        # [C2, DIM] -> [128, NB, ND, BS]
        xe8 = np.ascontiguousarray(
            xe.reshape(NB, BS, ND, 128).transpose(3, 0, 2, 1)).astype(f8)
        sl = slice(TS * e, TS * (e + 1))
        in_maps.append({
            "xe8": xe8,
            "xs": pack_x(np.ascontiguousarray(xt[sl].T).astype(f16), ND),
            "w1t8": pack_w((w1[e] * SW).T.astype(f8), DIM),
            "w3t8": pack_w((w3[e] * SW).T.astype(f8), DIM),
            "w2t8": pack_w((w2[e] * SW).T.astype(f8), INTER),
            "ws1t": ws1t, "ws3t": ws3t, "ws2t": ws2t,
            "biases": _pack_biases(b1[e], b3[e], b2[e], bs1, bs3, bs2),
        })

    key = (C2, BS, NB)
    if key not in _program_cache:
        _program_cache[key] = build_program(C2, BS, NB)
    nc = _program_cache[key]

    res = bass_utils.run_bass_kernel_spmd(
        nc, in_maps, core_ids=list(range(N_CORES)), trace=trace)

    y = np.empty((T, DIM), np.float32)
    for e in range(E):
        sl = slice(TS * e, TS * (e + 1))
        y[sl] = res.results[e]["ys"].T.astype(np.float32)
    for e in range(E):
        yef = res.results[e]["ye"].astype(np.float32)
        y[idx[e]] += yef[:, :len(idx[e])].T * wgt[e][:, None]
    # Spilled over-capacity pairs: exact fp32 on the host.
    for e, toks, we in spills:
        xs_ = xt[toks]
        h = _silu(xs_ @ w1[e].T + b1[e]) * (xs_ @ w3[e].T + b3[e])
        y[toks] += (h @ w2[e].T + b2[e]) * we[:, None]
    return y.reshape(B, S, DIM), res


def kernel(**inputs) -> np.ndarray:
    out, _ = run(inputs, trace=False)
    return out
